# revision 3
# baseline (speedup 1.0000x reference)
"""ALiBi (attention linear biases) kernel for Trainium2, 8 NeuronCores.

Problem: out = attention_scores + bias, where
  attention_scores: (2, 16, 2048, 2048) f32
  bias[h, j] = slopes[h] * (j - 2047)  (causal ALiBi row bias, broadcast
  over batch and query rows)

Sharding: 2 batches x 16 heads = 32 (batch, head) matrices, 4 per core
across 8 cores. Each core processes a (8192, 2048) f32 slab.

Structure (chosen from hw-loop slope measurements on this hardware):
  - dedicated DMA rings: ALL loads on the sync HWDGE ring, ALL stores on
    the scalar HWDGE ring. Interleaving loads+stores on one FIFO ring
    lets a store (blocked on its tile's adds) head-block independent
    loads; dedicated rings keep both directions streaming.
  - bias prologue on the gpsimd SWDGE queue so it never touches the two
    HWDGE rings and overlaps the first data loads.
  - per-core HBM-path DMA caps at ~324 GB/s with all 8 cores active (NC
    pairs share an HBM stack); the 128 MiB/core of load+store traffic
    makes ~410 us/exec the hardware floor. This kernel measures at that
    floor (copy-only control kernel: same time).
"""

import os
import sys

import numpy as np

# Defensive: make sure the concourse/axon stack resolves even if the
# grading environment lacks the usual PYTHONPATH entries.
for _p in (
    "/root/.axon_site",
    "/root/.axon_site/_ro/trn_rl_repo",
    "/root/.axon_site/_ro/pypackages",
    "/opt/trn_rl_repo",
):
    if os.path.isdir(_p) and _p not in sys.path:
        sys.path.append(_p)
os.environ.setdefault("JAX_PLATFORMS", "axon,cpu")

NUM_HEADS = 16
SEQ = 2048
BATCH = 2
N_CORES = 8
PAIRS = BATCH * NUM_HEADS            # 32 (batch, head) matrices
PAIRS_PER_CORE = PAIRS // N_CORES    # 4
ROWS_PER_CORE = PAIRS_PER_CORE * SEQ # 8192
P = 128                              # SBUF partitions
ROWS_PER_PART = 2                    # rows folded into the free dim
TILE_ROWS = P * ROWS_PER_PART        # 256 rows per tile (2 MiB)
DATA_BUFS = 10

_NC_CACHE = None


def _build_nc(rows_per_part=ROWS_PER_PART, bufs=DATA_BUFS, repeat=1,
              hw_loop=False, add_engines=("vector",)):
    import contextlib

    import concourse.bacc as bacc
    import concourse.mybir as mybir
    from concourse.tile import TileContext

    f32 = mybir.dt.float32
    # Bacc (not raw Bass): its compile() splits multi-sem waits into event
    # semaphores — TRN2 allows at most one sync wait per engine instruction.
    nc = bacc.Bacc()
    scores = nc.declare_dram_parameter(
        "scores", [ROWS_PER_CORE, SEQ], f32, isOutput=False
    )
    bias = nc.declare_dram_parameter(
        "bias", [PAIRS_PER_CORE, P, SEQ], f32, isOutput=False
    )
    out = nc.declare_dram_parameter("out", [ROWS_PER_CORE, SEQ], f32, isOutput=True)

    tile_rows = P * rows_per_part
    tiles_per_pair = SEQ // tile_rows
    n_tiles = ROWS_PER_CORE // tile_rows

    # Partition p of tile t holds rows t*tile_rows + p*rows_per_part ..
    # -> each partition reads a contiguous span from HBM; the whole tile
    # is one contiguous block.
    scores_v = scores.rearrange("(t p n) m -> t p (n m)", p=P, n=rows_per_part)
    out_v = out.rearrange("(t p n) m -> t p (n m)", p=P, n=rows_per_part)

    with TileContext(nc) as tc:
        with (
            tc.tile_pool(name="bias", bufs=1) as bias_pool,
            tc.tile_pool(name="data", bufs=bufs) as pool,
        ):
            bias_tiles = []
            for q in range(PAIRS_PER_CORE):
                bt = bias_pool.tile([P, SEQ], f32, tag=f"bias{q}")
                # gpsimd (SWDGE): keeps the bias prologue off the two
                # HWDGE rings so it overlaps the first data loads.
                nc.gpsimd.dma_start(out=bt[:], in_=bias[q])
                bias_tiles.append(bt)

            adders = [getattr(nc, a) for a in add_engines]
            if hw_loop:
                rep_ctx = tc.For_i(0, repeat)
                rep_range = [0]
            else:
                rep_ctx = contextlib.nullcontext()
                rep_range = range(repeat)
            with rep_ctx:
                for _rep in rep_range:
                    for t in range(n_tiles):
                        q = t // tiles_per_pair
                        tile = pool.tile([P, rows_per_part * SEQ], f32, tag="data")
                        nc.sync.dma_start(out=tile[:], in_=scores_v[t])
                        for k in range(rows_per_part):
                            a = adders[k % len(adders)]
                            a.tensor_add(
                                out=tile[:, k * SEQ : (k + 1) * SEQ],
                                in0=tile[:, k * SEQ : (k + 1) * SEQ],
                                in1=bias_tiles[q][:],
                            )
                        nc.scalar.dma_start(out=out_v[t], in_=tile[:])
    nc.compile()
    return nc


def _get_nc():
    global _NC_CACHE
    if _NC_CACHE is None:
        _NC_CACHE = _build_nc()
    return _NC_CACHE


def _alibi_bias_rows():
    """(NUM_HEADS, SEQ) f32: slopes[h] * (j - (SEQ-1)), matching reference."""
    ratio = 2.0 ** (-8.0 / NUM_HEADS)
    slopes = (ratio ** np.arange(1, 1 + NUM_HEADS, dtype=np.float64)).astype(
        np.float32
    )
    dist = np.arange(1 - SEQ, 1, dtype=np.float32)
    return slopes[:, None] * dist[None, :]


def _make_in_maps(attention_scores):
    x = np.ascontiguousarray(np.asarray(attention_scores), dtype=np.float32)
    assert x.shape == (BATCH, NUM_HEADS, SEQ, SEQ), x.shape
    flat = x.reshape(PAIRS, SEQ, SEQ)
    bias16 = _alibi_bias_rows()
    in_maps = []
    for c in range(N_CORES):
        lo = c * PAIRS_PER_CORE
        scores_c = flat[lo : lo + PAIRS_PER_CORE].reshape(ROWS_PER_CORE, SEQ)
        heads = [(lo + q) % NUM_HEADS for q in range(PAIRS_PER_CORE)]
        bias_c = np.ascontiguousarray(
            np.broadcast_to(
                bias16[heads][:, None, :], (PAIRS_PER_CORE, P, SEQ)
            ),
            dtype=np.float32,
        )
        in_maps.append({"scores": np.ascontiguousarray(scores_c), "bias": bias_c})
    return in_maps


def _run(in_maps, **kwargs):
    from concourse.bass_utils import run_bass_kernel_spmd

    return run_bass_kernel_spmd(
        _get_nc(), in_maps, core_ids=list(range(N_CORES)), **kwargs
    )


def _gather(results):
    out = np.concatenate(
        [np.asarray(r["out"]).reshape(PAIRS_PER_CORE, SEQ, SEQ) for r in results],
        axis=0,
    )
    return out.reshape(BATCH, NUM_HEADS, SEQ, SEQ)


def kernel(attention_scores):
    res = _run(_make_in_maps(attention_scores))
    return _gather(res.results)


# revision 8
# speedup vs baseline: 1.0118x; 1.0118x over previous
"""ALiBi (attention linear biases) kernel for Trainium2, 8 NeuronCores.

Problem: out = attention_scores + bias, where
  attention_scores: (2, 16, 2048, 2048) f32
  bias[h, j] = slopes[h] * (j - 2047)  (causal ALiBi row bias, broadcast
  over batch and query rows)

Sharding: 2 batches x 16 heads = 32 (batch, head) matrices, 4 per core
across 8 cores. Each core processes a (8192, 2048) f32 slab.

Structure (chosen from hw-loop slope measurements on this hardware):
  - dedicated DMA rings: ALL loads on the sync HWDGE ring, ALL stores on
    the scalar HWDGE ring. Interleaving loads+stores on one FIFO ring
    lets a store (blocked on its tile's adds) head-block independent
    loads; dedicated rings keep both directions streaming.
  - bias prologue: host sends only the 4 per-head bias rows (32 KB); the
    device broadcasts them across partitions with a K=1 ones-matmul
    (PE+PSUM), overlapped with the first data loads. Avoids 4 MiB of
    HBM bias traffic per execution.
  - per-core HBM-path DMA caps at ~324 GB/s with all 8 cores active (NC
    pairs share an HBM stack); the 128 MiB/core of load+store traffic
    makes ~410 us/exec the hardware floor. This kernel measures at that
    floor (copy-only control kernel: same time).
"""

import os
import sys

import numpy as np

# Defensive: make sure the concourse/axon stack resolves even if the
# grading environment lacks the usual PYTHONPATH entries.
for _p in (
    "/root/.axon_site",
    "/root/.axon_site/_ro/trn_rl_repo",
    "/root/.axon_site/_ro/pypackages",
    "/opt/trn_rl_repo",
):
    if os.path.isdir(_p) and _p not in sys.path:
        sys.path.append(_p)
os.environ.setdefault("JAX_PLATFORMS", "axon,cpu")

NUM_HEADS = 16
SEQ = 2048
BATCH = 2
N_CORES = 8
PAIRS = BATCH * NUM_HEADS            # 32 (batch, head) matrices
PAIRS_PER_CORE = PAIRS // N_CORES    # 4
ROWS_PER_CORE = PAIRS_PER_CORE * SEQ # 8192
P = 128                              # SBUF partitions
ROWS_PER_PART = 2                    # rows folded into the free dim
TILE_ROWS = P * ROWS_PER_PART        # 256 rows per tile (2 MiB)
DATA_BUFS = 8

_NC_CACHE = None


def _build_nc(rows_per_part=ROWS_PER_PART, bufs=DATA_BUFS, repeat=1,
              hw_loop=False, staggered=False, add_engines=("vector",),
              bias_on_device=True):
    import contextlib

    import concourse.bacc as bacc
    import concourse.mybir as mybir
    from concourse.tile import TileContext

    f32 = mybir.dt.float32
    # Bacc (not raw Bass): its compile() splits multi-sem waits into event
    # semaphores — TRN2 allows at most one sync wait per engine instruction.
    nc = bacc.Bacc()
    scores = nc.declare_dram_parameter(
        "scores", [ROWS_PER_CORE, SEQ], f32, isOutput=False
    )
    bias_shape = (
        [PAIRS_PER_CORE, SEQ] if bias_on_device else [PAIRS_PER_CORE, P, SEQ]
    )
    bias = nc.declare_dram_parameter("bias", bias_shape, f32, isOutput=False)
    out = nc.declare_dram_parameter("out", [ROWS_PER_CORE, SEQ], f32, isOutput=True)

    tile_rows = P * rows_per_part
    tiles_per_pair = SEQ // tile_rows
    n_tiles = ROWS_PER_CORE // tile_rows

    # Partition p of tile t holds rows t*tile_rows + p*rows_per_part ..
    # -> each partition reads a contiguous span from HBM; the whole tile
    # is one contiguous block.
    scores_v = scores.rearrange("(t p n) m -> t p (n m)", p=P, n=rows_per_part)
    out_v = out.rearrange("(t p n) m -> t p (n m)", p=P, n=rows_per_part)

    with TileContext(nc) as tc:
        with (
            tc.tile_pool(name="bias", bufs=1) as bias_pool,
            tc.tile_pool(name="data", bufs=bufs) as pool,
        ):
            bias_tiles = []
            if bias_on_device:
                # Host sends only the 4 bias rows (32 KB); broadcast each
                # across the 128 partitions with a K=1 ones-matmul into
                # PSUM. Saves the 4 MiB pre-broadcast bias DMA (~13 us of
                # HBM time on a single-shot execution); PE/DVE prologue
                # work hides under the first data loads.
                with (
                    tc.tile_pool(name="brow", bufs=1) as brow_pool,
                    tc.tile_pool(name="bpsum", bufs=2, space="PSUM") as psum_pool,
                ):
                    ones = brow_pool.tile([1, P], f32, tag="ones")
                    nc.gpsimd.memset(ones[:], 1.0)
                    for q in range(PAIRS_PER_CORE):
                        row = brow_pool.tile([1, SEQ], f32, tag=f"row{q}")
                        nc.gpsimd.dma_start(out=row[:], in_=bias[q : q + 1])
                        bt = bias_pool.tile([P, SEQ], f32, tag=f"bias{q}")
                        ps = psum_pool.tile([P, SEQ], f32, tag="ps")
                        for j in range(SEQ // 512):
                            nc.tensor.matmul(
                                ps[:, j * 512 : (j + 1) * 512],
                                ones[:],
                                row[0:1, j * 512 : (j + 1) * 512],
                            )
                        nc.vector.tensor_copy(out=bt[:], in_=ps[:])
                        bias_tiles.append(bt)
            else:
                for q in range(PAIRS_PER_CORE):
                    bt = bias_pool.tile([P, SEQ], f32, tag=f"bias{q}")
                    # gpsimd (SWDGE): keeps the bias prologue off the two
                    # HWDGE rings so it overlaps the first data loads.
                    nc.gpsimd.dma_start(out=bt[:], in_=bias[q])
                    bias_tiles.append(bt)

            adders = [getattr(nc, a) for a in add_engines]
            if hw_loop:
                rep_ctx = tc.For_i(0, repeat, staggered_reset=staggered)
                rep_range = [0]
            else:
                rep_ctx = contextlib.nullcontext()
                rep_range = range(repeat)
            with rep_ctx:
                for _rep in rep_range:
                    for t in range(n_tiles):
                        q = t // tiles_per_pair
                        tile = pool.tile([P, rows_per_part * SEQ], f32, tag="data")
                        nc.sync.dma_start(out=tile[:], in_=scores_v[t])
                        for k in range(rows_per_part):
                            a = adders[k % len(adders)]
                            a.tensor_add(
                                out=tile[:, k * SEQ : (k + 1) * SEQ],
                                in0=tile[:, k * SEQ : (k + 1) * SEQ],
                                in1=bias_tiles[q][:],
                            )
                        nc.scalar.dma_start(out=out_v[t], in_=tile[:])
    nc.compile()
    return nc


def _get_nc():
    global _NC_CACHE
    if _NC_CACHE is None:
        _NC_CACHE = _build_nc()
    return _NC_CACHE


def _alibi_bias_rows():
    """(NUM_HEADS, SEQ) f32: slopes[h] * (j - (SEQ-1)), matching reference."""
    ratio = 2.0 ** (-8.0 / NUM_HEADS)
    slopes = (ratio ** np.arange(1, 1 + NUM_HEADS, dtype=np.float64)).astype(
        np.float32
    )
    dist = np.arange(1 - SEQ, 1, dtype=np.float32)
    return slopes[:, None] * dist[None, :]


def _make_in_maps(attention_scores):
    x = np.ascontiguousarray(np.asarray(attention_scores), dtype=np.float32)
    assert x.shape == (BATCH, NUM_HEADS, SEQ, SEQ), x.shape
    flat = x.reshape(PAIRS, SEQ, SEQ)
    bias16 = _alibi_bias_rows()
    in_maps = []
    for c in range(N_CORES):
        lo = c * PAIRS_PER_CORE
        scores_c = flat[lo : lo + PAIRS_PER_CORE].reshape(ROWS_PER_CORE, SEQ)
        heads = [(lo + q) % NUM_HEADS for q in range(PAIRS_PER_CORE)]
        # device broadcasts across partitions (ones-matmul); host sends
        # just the per-head bias rows
        bias_c = np.ascontiguousarray(bias16[heads], dtype=np.float32)
        in_maps.append({"scores": np.ascontiguousarray(scores_c), "bias": bias_c})
    return in_maps


def _run(in_maps, **kwargs):
    from concourse.bass_utils import run_bass_kernel_spmd

    return run_bass_kernel_spmd(
        _get_nc(), in_maps, core_ids=list(range(N_CORES)), **kwargs
    )


def _gather(results):
    out = np.concatenate(
        [np.asarray(r["out"]).reshape(PAIRS_PER_CORE, SEQ, SEQ) for r in results],
        axis=0,
    )
    return out.reshape(BATCH, NUM_HEADS, SEQ, SEQ)


def kernel(attention_scores):
    res = _run(_make_in_maps(attention_scores))
    return _gather(res.results)
